# revision 29
# baseline (speedup 1.0000x reference)
"""CZ-ring (12 wires) applied to a batch of states: y = U @ x.

Every gate in the ring is a controlled-Z, which is diagonal in the
computational basis: CZ(c,t) = diag((-1)^(b_c & b_t)).  The product of
the 12 ring CZ gates is therefore also diagonal:

    U = diag(d),   d[b] = (-1)^(sum_i b_i * b_{(i+1) mod 12})

so U @ x is just a per-row sign flip of x.  Of the 4096 rows, 2112
have d=+1 and 1984 have d=-1.  Rows are sharded across the 8 cores
with a host-side permutation; data is moved in bfloat16 (max rel err
2^-8 = 0.39%, far inside the 2e-2 gate), which halves HBM traffic.

Per-core shard layout ([512, 1024] bf16, 1 MiB):
    rows   0..255  "neg block": 248 "-" rows + 8 "+" rows that are
                   pre-negated on the host (so a whole-block negate
                   leaves them unchanged).  One 512 KiB load on the
                   SP HWDGE ring into a [128, 2048] SBUF tile
                   (partition p holds rows 2p, 2p+1 -> 4 KiB fat
                   descriptors), one full-width vector-engine
                   multiply by -1.0, one 512 KiB store on the ACT
                   HWDGE ring.
    rows 256..511  "plus block": copied DRAM -> DRAM in one DMA on
                   the SP ring (no SBUF round trip, no dependencies).

The two HWDGE rings issue in parallel and the 16 SDMA engines
round-robin between them, so the load, d2d copy and store overlap;
per core ~2 MiB of HBM traffic total.  The runtime-injected NEFF
epilogue (full semaphore-file sweep, ~7 us) runs after the kernel
body regardless; with FINAL_WAIT=False the store/copy completion
latency is absorbed by it (NRT drains the DMA rings before
completing the NEFF, so the outputs are still guaranteed landed —
verified: store sems land ~7 us before NEFF end).
"""

import numpy as np
import ml_dtypes

N_WIRES = 12
DIM = 1 << N_WIRES  # 4096
BATCH = 1024
N_CORES = 8
ROWS_PER_CORE = DIM // N_CORES  # 512
P = 128
PLUS_PER_CORE = 264  # 2112 / 8
MINUS_PER_CORE = 248  # 1984 / 8
NEG_BLOCK = 2 * P  # 256 rows negated on device (248 "-" + 8 pre-negated "+")

# False: no engine waits on store/copy completion; the NEFF teardown's
# DMA ring drain (under the runtime-injected epilogue, ~7 us long)
# guarantees the data lands before execution completes.
FINAL_WAIT = False
# False: emit instructions straight into the main block, skipping the
# Block entry/exit all-engine barriers.
USE_BLOCK = False
# Strip the framework's const-pool MEMSETs (dead code for this program).
STRIP_MEMSETS = True
# Strip bass's post-preamble all-engine barrier (cross-engine deps in
# this program are fully covered by the data semaphores).
STRIP_BARRIER = False

_cache: dict = {}


def _sign_parity() -> np.ndarray:
    """parity[b] = sum_i b_i * b_{(i+1) mod N_WIRES} mod 2  (1 => d=-1)."""
    b = np.arange(DIM, dtype=np.uint32)
    parity = np.zeros(DIM, dtype=np.uint32)
    for i in range(N_WIRES):
        bi = (b >> np.uint32(i)) & np.uint32(1)
        bj = (b >> np.uint32((i + 1) % N_WIRES)) & np.uint32(1)
        parity ^= bi & bj
    return parity


def _row_assignment():
    """Per-core row index lists: [248 minus | 8 plus (pre-negated) | 256 plus]."""
    parity = _sign_parity()
    plus_rows = np.nonzero(parity == 0)[0]  # 2112
    minus_rows = np.nonzero(parity == 1)[0]  # 1984
    assert len(plus_rows) == PLUS_PER_CORE * N_CORES
    assert len(minus_rows) == MINUS_PER_CORE * N_CORES
    perms = []
    for k in range(N_CORES):
        p = plus_rows[k * PLUS_PER_CORE : (k + 1) * PLUS_PER_CORE]
        m = minus_rows[k * MINUS_PER_CORE : (k + 1) * MINUS_PER_CORE]
        perms.append(np.concatenate([m, p[NEG_BLOCK - MINUS_PER_CORE :], p[: NEG_BLOCK - MINUS_PER_CORE]]))
    return perms


def _build_program(
    final_wait: bool,
    use_block: bool = True,
    strip_memsets: bool = False,
    strip_barrier: bool = False,
):
    from concourse import bass
    import concourse.mybir as mybir

    bf16 = mybir.dt.bfloat16
    nc = bass.Bass(
        "TRN2", target_bir_lowering=False, debug=False, monotonic_sem_count=0
    )
    x_in = nc.dram_tensor("x", [ROWS_PER_CORE, BATCH], bf16, kind="ExternalInput").ap()
    y_out = nc.dram_tensor(
        "y", [ROWS_PER_CORE, BATCH], bf16, kind="ExternalOutput"
    ).ap()
    # One tile holding the whole neg block: partition p = rows 2p, 2p+1
    # (4 KiB contiguous DRAM per partition -> 128 fat descriptors).
    t = nc.alloc_sbuf_tensor("t", [P, 2 * BATCH], bf16).ap()

    # Raw bass (no TileContext): explicit standalone waits keep every
    # instruction at <=1 sync wait.
    ld = nc.alloc_semaphore("ld")
    dve = nc.alloc_semaphore("dve")
    st = nc.alloc_semaphore("st")
    pd = nc.alloc_semaphore("pd")

    x_neg = x_in[:NEG_BLOCK, :].rearrange("(p n) d -> p (n d)", p=P)
    y_neg = y_out[:NEG_BLOCK, :].rearrange("(p n) d -> p (n d)", p=P)

    def emit_sync(sync: bass.BassEngine):
        # One load for the whole neg block, then the independent
        # plus-block DRAM->DRAM copy streams behind it on the SP ring.
        # Every DMA carries a sem inc (walrus requires sync info on DGE
        # ops); with final_wait=False the store/copy sems are simply
        # never waited on — the NEFF teardown's ring drain covers the
        # data landing.
        sync.dma_start(out=t, in_=x_neg).then_inc(ld, 16)
        sync.dma_start(out=y_out[NEG_BLOCK:, :], in_=x_in[NEG_BLOCK:, :]).then_inc(pd, 16)
        if final_wait:
            sync.wait_ge(pd, 16)

    def emit_vector(vector: bass.BassEngine):
        # single full-width negate of the whole block once it's resident.
        # (Splitting this with an ACT-engine activation(scale=-1) was
        # tried and is both slow — ACT table-load/startup — and racy:
        # engine program order does NOT order the ACT datapath against
        # the store's SDMA reads without an extra semaphore.)
        vector.wait_ge(ld, 16)
        vector.tensor_scalar_mul(t, t, -1.0).then_inc(dve, 1)

    def emit_scalar(scalar: bass.BassEngine):
        scalar.wait_ge(dve, 1)
        scalar.dma_start(out=y_neg, in_=t).then_inc(st, 16)
        if final_wait:
            scalar.wait_ge(st, 16)

    if use_block:
        with nc.Block() as block:
            block.sync(emit_sync)
            block.vector(emit_vector)
            block.scalar(emit_scalar)
    else:
        # Direct emission into main: skips the Block entry/exit barriers
        # (the NEFF-level all-engine barrier before the epilogue still
        # orders everything; cross-engine deps are covered by the sems).
        emit_sync(nc.sync)
        emit_vector(nc.vector)
        emit_scalar(nc.scalar)

    bb = nc.main_func.blocks[0]
    if strip_memsets:
        # The const-pool tiles these initialize are never referenced by
        # this program (the negate uses an immediate).
        for m in [i for i in bb.instructions if type(i).__name__ == "InstMemset"]:
            bb.instructions.remove(m)
    if strip_barrier:
        for m in [i for i in bb.instructions if "barrier_" in str(i)]:
            bb.instructions.remove(m)

    return nc


def kernel(x: np.ndarray, **trace_kwargs) -> np.ndarray:
    from concourse.bass_utils import run_bass_kernel_spmd

    x = np.asarray(x, dtype=np.float32)
    key = ("nc", FINAL_WAIT, USE_BLOCK, STRIP_MEMSETS, STRIP_BARRIER)
    if key not in _cache:
        _cache[key] = _build_program(FINAL_WAIT, USE_BLOCK, STRIP_MEMSETS, STRIP_BARRIER)
        _cache.setdefault("perms", _row_assignment())
    nc = _cache[key]
    perms = _cache["perms"]

    in_maps = []
    for perm in perms:
        xs = x[perm]
        # the neg block holds 8 "+" rows (shard positions 248..255); the
        # device negates the block wholesale, so pre-negate to compensate
        xs[MINUS_PER_CORE:NEG_BLOCK] *= -1.0
        in_maps.append({"x": xs.astype(ml_dtypes.bfloat16)})

    res = run_bass_kernel_spmd(
        nc, in_maps, core_ids=list(range(N_CORES)), **trace_kwargs
    )
    _cache["last_results"] = res

    y = np.empty((DIM, BATCH), dtype=np.float32)
    for perm, r in zip(perms, res.results):
        y[perm] = r["y"].astype(np.float32)
    return y


# revision 30
# speedup vs baseline: 1.0174x; 1.0174x over previous
"""CZ-ring (12 wires) applied to a batch of states: y = U @ x.

Every gate in the ring is a controlled-Z, which is diagonal in the
computational basis: CZ(c,t) = diag((-1)^(b_c & b_t)).  The product of
the 12 ring CZ gates is therefore also diagonal:

    U = diag(d),   d[b] = (-1)^(sum_i b_i * b_{(i+1) mod 12})

so U @ x is just a per-row sign flip of x.  Of the 4096 rows, 2112
have d=+1 and 1984 have d=-1.  Rows are sharded across the 8 cores
with a host-side permutation; data is moved in bfloat16 (max rel err
2^-8 = 0.39%, far inside the 2e-2 gate), which halves HBM traffic.

Per-core shard layout ([512, 1024] bf16, 1 MiB):
    rows   0..255  "neg block": 248 "-" rows + 8 "+" rows that are
                   pre-negated on the host (so a whole-block negate
                   leaves them unchanged).  One 512 KiB load on the
                   SP HWDGE ring into a [128, 2048] SBUF tile
                   (partition p holds rows 2p, 2p+1 -> 4 KiB fat
                   descriptors), one full-width vector-engine
                   multiply by -1.0, one 512 KiB store on the ACT
                   HWDGE ring.
    rows 256..511  "plus block": copied DRAM -> DRAM in one DMA on
                   the SP ring (no SBUF round trip, no dependencies).

The two HWDGE rings issue in parallel and the 16 SDMA engines
round-robin between them, so the load, d2d copy and store overlap;
per core ~2 MiB of HBM traffic total.  The runtime-injected NEFF
epilogue (full semaphore-file sweep, ~7 us) runs after the kernel
body regardless; with FINAL_WAIT=False the store/copy completion
latency is absorbed by it (NRT drains the DMA rings before
completing the NEFF, so the outputs are still guaranteed landed —
verified: store sems land ~7 us before NEFF end).
"""

import numpy as np
import ml_dtypes

N_WIRES = 12
DIM = 1 << N_WIRES  # 4096
BATCH = 1024
N_CORES = 8
ROWS_PER_CORE = DIM // N_CORES  # 512
P = 128
PLUS_PER_CORE = 264  # 2112 / 8
MINUS_PER_CORE = 248  # 1984 / 8
NEG_BLOCK = 2 * P  # 256 rows negated on device (248 "-" + 8 pre-negated "+")

# False: no engine waits on store/copy completion; the NEFF teardown's
# DMA ring drain (under the runtime-injected epilogue, ~7 us long)
# guarantees the data lands before execution completes.
FINAL_WAIT = False
# False: emit instructions straight into the main block, skipping the
# Block entry/exit all-engine barriers.
USE_BLOCK = False
# Strip the framework's const-pool MEMSETs (dead code for this program).
STRIP_MEMSETS = True
# Strip bass's post-preamble all-engine barrier (cross-engine deps in
# this program are fully covered by the data semaphores).
STRIP_BARRIER = False

_cache: dict = {}


def _sign_parity() -> np.ndarray:
    """parity[b] = sum_i b_i * b_{(i+1) mod N_WIRES} mod 2  (1 => d=-1)."""
    b = np.arange(DIM, dtype=np.uint32)
    parity = np.zeros(DIM, dtype=np.uint32)
    for i in range(N_WIRES):
        bi = (b >> np.uint32(i)) & np.uint32(1)
        bj = (b >> np.uint32((i + 1) % N_WIRES)) & np.uint32(1)
        parity ^= bi & bj
    return parity


def _row_assignment():
    """Per-core row index lists: [248 minus | 8 plus (pre-negated) | 256 plus]."""
    parity = _sign_parity()
    plus_rows = np.nonzero(parity == 0)[0]  # 2112
    minus_rows = np.nonzero(parity == 1)[0]  # 1984
    assert len(plus_rows) == PLUS_PER_CORE * N_CORES
    assert len(minus_rows) == MINUS_PER_CORE * N_CORES
    perms = []
    for k in range(N_CORES):
        p = plus_rows[k * PLUS_PER_CORE : (k + 1) * PLUS_PER_CORE]
        m = minus_rows[k * MINUS_PER_CORE : (k + 1) * MINUS_PER_CORE]
        perms.append(np.concatenate([m, p[NEG_BLOCK - MINUS_PER_CORE :], p[: NEG_BLOCK - MINUS_PER_CORE]]))
    return perms


def _build_program(
    final_wait: bool,
    use_block: bool = True,
    strip_memsets: bool = False,
    strip_barrier: bool = False,
):
    from concourse import bass
    import concourse.mybir as mybir

    bf16 = mybir.dt.bfloat16
    nc = bass.Bass(
        "TRN2", target_bir_lowering=False, debug=False, monotonic_sem_count=0
    )
    x_in = nc.dram_tensor("x", [ROWS_PER_CORE, BATCH], bf16, kind="ExternalInput").ap()
    y_out = nc.dram_tensor(
        "y", [ROWS_PER_CORE, BATCH], bf16, kind="ExternalOutput"
    ).ap()
    # One tile holding the whole neg block: partition p = rows 2p, 2p+1
    # (4 KiB contiguous DRAM per partition -> 128 fat descriptors).
    t = nc.alloc_sbuf_tensor("t", [P, 2 * BATCH], bf16).ap()

    # Raw bass (no TileContext): explicit standalone waits keep every
    # instruction at <=1 sync wait.
    ld = nc.alloc_semaphore("ld")
    dve = nc.alloc_semaphore("dve")
    st = nc.alloc_semaphore("st")
    pd = nc.alloc_semaphore("pd")

    x_neg = x_in[:NEG_BLOCK, :].rearrange("(p n) d -> p (n d)", p=P)
    y_neg = y_out[:NEG_BLOCK, :].rearrange("(p n) d -> p (n d)", p=P)

    def emit_sync(sync: bass.BassEngine):
        # One load for the whole neg block, then the independent
        # plus-block DRAM->DRAM copy streams behind it on the SP ring.
        # Every DMA carries a sem inc (walrus requires sync info on DGE
        # ops); with final_wait=False the store/copy sems are simply
        # never waited on — the NEFF teardown's ring drain covers the
        # data landing.
        sync.dma_start(out=t, in_=x_neg).then_inc(ld, 16)
        sync.dma_start(out=y_out[NEG_BLOCK:, :], in_=x_in[NEG_BLOCK:, :]).then_inc(pd, 16)
        if final_wait:
            sync.wait_ge(pd, 16)

    def emit_vector(vector: bass.BassEngine):
        # single full-width negate of the whole block once it's resident.
        # (Splitting this with an ACT-engine activation(scale=-1) was
        # tried and is both slow — ACT table-load/startup — and racy:
        # engine program order does NOT order the ACT datapath against
        # the store's SDMA reads without an extra semaphore.)
        vector.wait_ge(ld, 16)
        vector.tensor_scalar_mul(t, t, -1.0).then_inc(dve, 1)

    def emit_scalar(scalar: bass.BassEngine):
        # the dve wait is fused onto the DMA instruction itself — saves
        # the standalone wait's dispatch on the post-negate critical path
        scalar.dma_start(out=y_neg, in_=t).then_inc(st, 16).wait_op(dve, 1, "sem-ge")
        if final_wait:
            scalar.wait_ge(st, 16)

    if use_block:
        with nc.Block() as block:
            block.sync(emit_sync)
            block.vector(emit_vector)
            block.scalar(emit_scalar)
    else:
        # Direct emission into main: skips the Block entry/exit barriers
        # (the NEFF-level all-engine barrier before the epilogue still
        # orders everything; cross-engine deps are covered by the sems).
        emit_sync(nc.sync)
        emit_vector(nc.vector)
        emit_scalar(nc.scalar)

    bb = nc.main_func.blocks[0]
    if strip_memsets:
        # The const-pool tiles these initialize are never referenced by
        # this program (the negate uses an immediate).
        for m in [i for i in bb.instructions if type(i).__name__ == "InstMemset"]:
            bb.instructions.remove(m)
    if strip_barrier:
        for m in [i for i in bb.instructions if "barrier_" in str(i)]:
            bb.instructions.remove(m)

    return nc


def kernel(x: np.ndarray, **trace_kwargs) -> np.ndarray:
    from concourse.bass_utils import run_bass_kernel_spmd

    x = np.asarray(x, dtype=np.float32)
    key = ("nc", FINAL_WAIT, USE_BLOCK, STRIP_MEMSETS, STRIP_BARRIER)
    if key not in _cache:
        _cache[key] = _build_program(FINAL_WAIT, USE_BLOCK, STRIP_MEMSETS, STRIP_BARRIER)
        _cache.setdefault("perms", _row_assignment())
    nc = _cache[key]
    perms = _cache["perms"]

    in_maps = []
    for perm in perms:
        xs = x[perm]
        # the neg block holds 8 "+" rows (shard positions 248..255); the
        # device negates the block wholesale, so pre-negate to compensate
        xs[MINUS_PER_CORE:NEG_BLOCK] *= -1.0
        in_maps.append({"x": xs.astype(ml_dtypes.bfloat16)})

    res = run_bass_kernel_spmd(
        nc, in_maps, core_ids=list(range(N_CORES)), **trace_kwargs
    )
    _cache["last_results"] = res

    y = np.empty((DIM, BATCH), dtype=np.float32)
    for perm, r in zip(perms, res.results):
        y[perm] = r["y"].astype(np.float32)
    return y
